# revision 20
# baseline (speedup 1.0000x reference)
"""Trainium2 Bass kernel: BlockAttnRes forward.

Reference computation (per batch b, position t):
    k[n]   = s[n] / sqrt(mean(s[n]^2) + eps)        n in [0, 9)
    score  = k[n] . w                                (w = queries[layer_idx])
    alpha  = softmax(score over n)
    h[t]   = sum_n alpha[n] * s[n]                   (d = 512)

Distribution: batch dim B=8 -> one batch per NeuronCore (8 cores), no
cross-core communication.  Per core: T=4096 positions processed in 32
tiles of 128 (partition dim = position).

Engine budget per tile (target: DMA-bound ~7.3us/tile):
    DMA  : 3 chunk loads [128,3,512] f32 (6.55us) + out write (0.73us)
    ACT  : 9x Square+accum (ssq), Ln, Exp (rsq = 1/rms), Exp (e, accum
           gives sum_e free), PSUM->SBUF drain copy       ~7.5us
    DVE  : 9x STT dot+accum, score=dot*rsq, -max, 1/sum_e,
           dg = (I*rs)*e fused diag build (one STT)       ~7.7us
    PE   : h_psum += diag.T @ s_n in float32r (1 cyc/row;
           plain fp32 is 4 cyc/row)                       ~4us

The emission is software-pipelined with per-phase lags so every
cross-engine dependency is produced a full tile-period before its
consumer: each engine's in-order queue then never parks (head-of-line
waits were the main loss in the naive per-tile ordering).  Schedule at
iteration c:
    SP  : loads(c)                                   ... out(c-3)
    ACT : ln/rsq(c-1), e(c-2), 9x sq(c), drain(c-3)
    DVE : dot0(c), score/nmx(c-1), dots1-8(c), recip(c-2), dg(c-2)
    PE  : matmuls(c-2)

All ACT functions (square, ln, exp, copy) live in the single
`natural_log_exp_and_others` table set -> one ACT_TABLE_LOAD total
(pinned via PinnedBacc below; the stock chooser thrashes sets).
"""

import numpy as np

B, T, N, D = 8, 4096, 9, 512
P = 128
EPS = 1e-6
NCORES = 8

_CACHE = {}


def _build_bass(t_len=T, n_chunks=3):
    import concourse.bass as bass
    import concourse.tile as tile
    from concourse import bacc, mybir

    f32 = mybir.dt.float32
    f32r = mybir.dt.float32r
    Alu = mybir.AluOpType
    Act = mybir.ActivationFunctionType
    Ax = mybir.AxisListType

    ntiles = t_len // P

    # Bacc (not raw Bass): its compile() pass splits multi-sem waits into
    # InstEventSemaphore — TRN2 engine instructions hold at most ONE wait.
    PINNED_SET = "natural_log_exp_and_others"

    class PinnedBacc(bacc.Bacc):
        def insert_act_table_loads(self):
            import bass_rust as _bass_rust
            from concourse.hw_specs import get_activation_tables

            all_tables = get_activation_tables(self.m.arch)
            used = {
                i.func
                for b in self.main_func.blocks
                for i in b.instructions
                if isinstance(i, mybir.InstActivation)
            }
            if used and PINNED_SET in all_tables and used <= all_tables[PINNED_SET]:
                tables = [
                    (name, funcs if name == PINNED_SET else set())
                    for name, funcs in all_tables.items()
                ]
            else:
                tables = list(all_tables.items())
            _bass_rust.insert_act_table_loads(self, tables)

    nc = PinnedBacc("TRN2", target_bir_lowering=False, debug=False)
    src = nc.dram_tensor("src", [t_len, N, D], f32, kind="ExternalInput").ap()
    wq = nc.dram_tensor("wq", [P, D], f32, kind="ExternalInput").ap()
    idn = nc.dram_tensor("idn", [P, P], f32, kind="ExternalInput").ap()
    out = nc.dram_tensor("out", [t_len, D], f32, kind="ExternalOutput").ap()

    src_t = src.rearrange("(c p) n d -> c p n d", p=P)
    out_t = out.rearrange("(c p) d -> c p d", p=P)

    assert N % n_chunks == 0
    cn = N // n_chunks  # n's per chunk

    with tile.TileContext(nc) as tc:
        with (
            tc.tile_pool(name="const", bufs=1) as const_pool,
            tc.tile_pool(name="srcp", bufs=7) as src_pool,
            tc.tile_pool(name="scratch", bufs=4) as scr_pool,
            tc.tile_pool(name="small", bufs=8) as small_pool,
            tc.tile_pool(name="diag", bufs=4) as diag_pool,
            tc.tile_pool(name="hout", bufs=4) as out_pool,
            tc.tile_pool(name="psum", bufs=8, space="PSUM") as psum_pool,
        ):
            w_sb = const_pool.tile([P, D], f32)
            nc.sync.dma_start(out=w_sb, in_=wq)
            i_sb = const_pool.tile([P, P], f32)
            nc.sync.dma_start(out=i_sb, in_=idn)
            eps_sb = const_pool.tile([P, 1], f32)
            nc.vector.memset(eps_sb, EPS)

            st = {}  # per-tile state: c -> dict of tiles

            def p_load(c):
                chunks = []
                for k in range(n_chunks):
                    sk = src_pool.tile([P, cn, D], f32r, tag=f"s{k}")
                    nc.sync.dma_start(
                        out=sk,
                        in_=src_t[c, :, k * cn : (k + 1) * cn, :].bitcast(f32r),
                    )
                    chunks.append(sk)
                st[c] = {"chunks": chunks}

            def s_mm(c, n):
                return st[c]["chunks"][n // cn][:, n % cn, :]

            def s_of(c, n):
                return s_mm(c, n).bitcast(f32)

            def p_sq(c):
                # squares n = 0..7 on ACT; n = 8 rides on DVE (p_sqd) to
                # balance the engines (ACT accum reads cost ~280ns each).
                ssq = small_pool.tile([P, N], f32, tag="ssq")
                sq = scr_pool.tile([P, D], f32, tag="sq")
                for n in range(N - 1):
                    nc.scalar.activation(
                        out=sq,
                        in_=s_of(c, n),
                        func=Act.Square,
                        accum_out=ssq[:, n : n + 1],
                    )
                st[c]["ssq"] = ssq

            def p_sqd(c):
                n = N - 1
                nc.vector.scalar_tensor_tensor(
                    out=st[c]["prod"],
                    in0=s_of(c, n),
                    scalar=0.0,
                    in1=s_of(c, n),
                    op0=Alu.bypass,
                    op1=Alu.mult,
                    accum_out=st[c]["ssq"][:, n : n + 1],
                )

            def p_ln(c):
                # rsq = (ssq/D + eps)^(-1/2) via Exp(-0.5 * Ln(x))
                rsq = small_pool.tile([P, N], f32, tag="rsq")
                nc.scalar.activation(
                    out=rsq, in_=st[c]["ssq"], func=Act.Ln, scale=1.0 / D, bias=eps_sb
                )
                nc.scalar.activation(out=rsq, in_=rsq, func=Act.Exp, scale=-0.5)
                st[c]["rsq"] = rsq

            def p_dot(c, lo, hi):
                if lo == 0:
                    st[c]["dot"] = small_pool.tile([P, N], f32, tag="dot", name="dot")
                    st[c]["prod"] = scr_pool.tile([P, D], f32, tag="prod", name="prod")
                dot, prod = st[c]["dot"], st[c]["prod"]
                for n in range(lo, hi):
                    nc.vector.scalar_tensor_tensor(
                        out=prod,
                        in0=s_of(c, n),
                        scalar=0.0,
                        in1=w_sb,
                        op0=Alu.bypass,
                        op1=Alu.mult,
                        accum_out=dot[:, n : n + 1],
                    )

            def p_score(c):
                # negscore = -(dot * rsq); negmax = -max(score)
                negscore = small_pool.tile([P, N], f32, tag="score")
                nc.vector.scalar_tensor_tensor(
                    out=negscore,
                    in0=st[c]["dot"],
                    scalar=-1.0,
                    in1=st[c]["rsq"],
                    op0=Alu.mult,
                    op1=Alu.mult,
                )
                nmx = small_pool.tile([P, 1], f32, tag="nmx")
                nc.vector.tensor_reduce(
                    out=nmx, in_=negscore, axis=Ax.X, op=Alu.min
                )
                st[c]["score"], st[c]["nmx"] = negscore, nmx

            def p_e(c):
                # e = Exp(score - max) = Exp(-negscore + negmax); unnormalized
                # — the 1/sum_e normalization rides on the drain's scale.
                # accum_out gives sum_e for free.
                e = small_pool.tile([P, N], f32, tag="e")
                sume = small_pool.tile([P, 1], f32, tag="sume")
                nc.scalar.activation(
                    out=e,
                    in_=st[c]["score"],
                    func=Act.Exp,
                    scale=-1.0,
                    bias=st[c]["nmx"],
                    accum_out=sume,
                )
                st[c]["e"], st[c]["sume"] = e, sume

            def p_sums(c):
                rs = small_pool.tile([P, 1], f32, tag="rs")
                nc.vector.reciprocal(out=rs, in_=st[c]["sume"])
                st[c]["rs"] = rs

            def p_dg(c):
                # dg[:, n, :] = I * e[:, n] = diag(e_n) — ONE broadcast
                # tensor_tensor: in0 = I broadcast over n (stride-0), in1 =
                # e broadcast over the 128 columns (stride-0 innermost).
                # f32r so the PE consumes it directly.  (gpsimd was tried
                # here and steals DVE SBUF-port bandwidth ~1:1 — keep on DVE.)
                dg = diag_pool.tile([P, N, P], f32r, tag="dg")
                i_sb_, e_ = i_sb, st[c]["e"]
                i_b = bass.AP(
                    tensor=i_sb_.tensor,
                    offset=i_sb_.offset,
                    ap=[i_sb_.ap[0], [0, N], i_sb_.ap[1]],
                )
                e_b = bass.AP(
                    tensor=e_.tensor,
                    offset=e_.offset,
                    ap=[e_.ap[0], [e_.ap[1][0], N], [0, P]],
                )
                nc.vector.tensor_mul(dg, i_b, e_b)
                st[c]["dg"] = dg

            def p_mm(c):
                hp = psum_pool.tile([P, D], f32, tag="hp")
                dg = st[c]["dg"]
                for n in range(N):
                    nc.tensor.matmul(
                        hp,
                        dg[:, n, :],
                        s_mm(c, n),
                        start=(n == 0),
                        stop=(n == N - 1),
                    )
                st[c]["hp"] = hp

            def p_drain(c):
                # h = hp / sum_e: the PSUM->SBUF move doubles as the softmax
                # normalization (per-partition scale).
                hs = out_pool.tile([P, D], f32, tag="hs")
                nc.scalar.activation(
                    out=hs, in_=st[c]["hp"], func=Act.Copy, scale=st[c]["rs"]
                )
                st[c]["hs"] = hs

            def p_out(c):
                nc.sync.dma_start(out=out_t[c], in_=st[c]["hs"])
                del st[c]  # release python refs; pool rotation handles bufs

            # Software-pipelined emission (see module docstring).  Loads run
            # 2 tiles ahead of their consumers; the out-DMA trails its drain
            # by a full tile so the SP queue never parks on a compute sem.
            for c in range(ntiles + 6):
                if c - 6 >= 0 and c - 6 < ntiles:
                    p_out(c - 6)
                if c < ntiles:
                    p_load(c)
                x = c - 3
                if 0 <= x < ntiles:
                    p_ln(x)
                x = c - 4
                if 0 <= x < ntiles:
                    p_e(x)
                x = c - 2
                if 0 <= x < ntiles:
                    p_dot(x, 0, 1)
                x = c - 3
                if 0 <= x < ntiles:
                    p_score(x)
                x = c - 2
                if 0 <= x < ntiles:
                    p_dot(x, 1, N)
                    p_sq(x)
                    p_sqd(x)
                x = c - 4
                if 0 <= x < ntiles:
                    p_sums(x)
                    p_dg(x)
                    p_mm(x)
                x = c - 5
                if 0 <= x < ntiles:
                    p_drain(x)

    nc.compile()
    return nc


def _get_nc(t_len=T):
    key = (t_len,)
    if key not in _CACHE:
        _CACHE[key] = _build_bass(t_len)
    return _CACHE[key]


def _make_in_maps(sources, queries, layer_idx):
    sources = np.ascontiguousarray(np.asarray(sources, dtype=np.float32))
    queries = np.asarray(queries, dtype=np.float32)
    w = queries[int(layer_idx)]
    w_rep = np.ascontiguousarray(np.broadcast_to(w[None, :], (P, D)).astype(np.float32))
    idn = np.eye(P, dtype=np.float32)
    return [
        {"src": np.ascontiguousarray(sources[b]), "wq": w_rep, "idn": idn}
        for b in range(sources.shape[0])
    ]


def kernel(sources, queries, layer_idx):
    from concourse.bass_utils import run_bass_kernel_spmd

    nc = _get_nc()
    in_maps = _make_in_maps(sources, queries, layer_idx)
    res = run_bass_kernel_spmd(nc, in_maps, core_ids=list(range(NCORES)))
    return np.stack([res.results[b]["out"] for b in range(NCORES)], axis=0)


# revision 23
# speedup vs baseline: 1.1944x; 1.1944x over previous
"""Trainium2 Bass kernel: BlockAttnRes forward.

Reference computation (per batch b, position t):
    k[n]   = s[n] / sqrt(mean(s[n]^2) + eps)        n in [0, 9)
    score  = k[n] . w                                (w = queries[layer_idx])
    alpha  = softmax(score over n)
    h[t]   = sum_n alpha[n] * s[n]                   (d = 512)

Distribution: batch dim B=8 -> one batch per NeuronCore (8 cores), no
cross-core communication.  Per core: T=4096 positions processed in 32
tiles of 128 (partition dim = position).

Engine budget per tile, all measured on HW (target: DMA ~7.3us/tile):
    DMA  : 3 chunk loads [128,3,512] f32 (6.55us) + out write (0.73us)
    ACT  : 8x Square+accum (~800ns each incl. serialized accum-register
           read), Ln + Exp (rsq = 1/rms), Exp (e, accum gives sum_e
           free), PSUM->SBUF drain copy with 1/sum_e scale   ~7.7us
    DVE  : 9x STT dot+accum (~612ns each), 1x STT square (n=8),
           negscore STT, -max reduce, 1/sum_e, dg = I*e broadcast
           diag build (~1.3us)                               ~7.75us
    PE   : h_psum += diag(e_n).T @ s_n in float32r (1 cyc/row at
           free-dim >= 256; plain fp32 is 4 cyc/row)         ~4us

ACT and DVE are the (balanced) walls; moving any op between them makes
the max worse.  The softmax is computed unnormalized (dg holds e, not
alpha) and the drain's per-partition scale applies 1/sum_e.  e stays
fp32: scores have std ~22, so even bf16 dot inputs (~0.3 absolute
score error) get amplified by exp into flipped alphas (measured 7e-2
rel err — fails).

The emission is software-pipelined with per-phase lags so every
cross-engine dependency is produced a full tile-period before its
consumer: each engine's in-order queue then never parks (head-of-line
waits were the main loss in the naive per-tile ordering).  Schedule at
iteration c (x = tile the phase applies to):
    SP  : out(c-6), loads(c)
    ACT : ln/rsq(c-3), e(c-4), 8x sq(c-2), drain(c-5)
    DVE : dot0(c-2), negscore/nmx(c-3), dots1-8(c-2), sq8(c-2),
          recip(c-4), dg(c-4)
    PE  : matmuls(c-4)
Steady state measured: DVE 100% busy, ACT ~97%; ~30us ramp/tail.

All ACT functions (square, ln, exp, copy) live in the single
`natural_log_exp_and_others` table set -> one ACT_TABLE_LOAD total
(pinned via PinnedBacc below; the stock chooser thrashes sets).
"""

import numpy as np

B, T, N, D = 8, 4096, 9, 512
P = 128
EPS = 1e-6
NCORES = 8

_CACHE = {}


def _build_bass(t_len=T, n_chunks=3):
    import concourse.bass as bass
    import concourse.tile as tile
    from concourse import bacc, mybir

    f32 = mybir.dt.float32
    f32r = mybir.dt.float32r
    Alu = mybir.AluOpType
    Act = mybir.ActivationFunctionType
    Ax = mybir.AxisListType

    ntiles = t_len // P

    # Bacc (not raw Bass): its compile() pass splits multi-sem waits into
    # InstEventSemaphore — TRN2 engine instructions hold at most ONE wait.
    PINNED_SET = "natural_log_exp_and_others"

    class PinnedBacc(bacc.Bacc):
        def insert_act_table_loads(self):
            import bass_rust as _bass_rust
            from concourse.hw_specs import get_activation_tables

            all_tables = get_activation_tables(self.m.arch)
            used = {
                i.func
                for b in self.main_func.blocks
                for i in b.instructions
                if isinstance(i, mybir.InstActivation)
            }
            if used and PINNED_SET in all_tables and used <= all_tables[PINNED_SET]:
                tables = [
                    (name, funcs if name == PINNED_SET else set())
                    for name, funcs in all_tables.items()
                ]
            else:
                tables = list(all_tables.items())
            _bass_rust.insert_act_table_loads(self, tables)

    nc = PinnedBacc("TRN2", target_bir_lowering=False, debug=False)
    src = nc.dram_tensor("src", [t_len, N, D], f32, kind="ExternalInput").ap()
    wq = nc.dram_tensor("wq", [P, D], f32, kind="ExternalInput").ap()
    idn = nc.dram_tensor("idn", [P, P], f32, kind="ExternalInput").ap()
    out = nc.dram_tensor("out", [t_len, D], f32, kind="ExternalOutput").ap()

    src_t = src.rearrange("(c p) n d -> c p n d", p=P)
    out_t = out.rearrange("(c p) d -> c p d", p=P)

    assert N % n_chunks == 0
    cn = N // n_chunks  # n's per chunk

    with tile.TileContext(nc) as tc:
        with (
            tc.tile_pool(name="const", bufs=1) as const_pool,
            tc.tile_pool(name="srcp", bufs=7) as src_pool,
            tc.tile_pool(name="scratch", bufs=4) as scr_pool,
            tc.tile_pool(name="small", bufs=8) as small_pool,
            tc.tile_pool(name="diag", bufs=4) as diag_pool,
            tc.tile_pool(name="hout", bufs=4) as out_pool,
            tc.tile_pool(name="psum", bufs=8, space="PSUM") as psum_pool,
        ):
            w_sb = const_pool.tile([P, D], f32)
            nc.sync.dma_start(out=w_sb, in_=wq)
            i_sb = const_pool.tile([P, P], f32)
            nc.sync.dma_start(out=i_sb, in_=idn)
            eps_sb = const_pool.tile([P, 1], f32)
            nc.vector.memset(eps_sb, EPS)

            st = {}  # per-tile state: c -> dict of tiles

            def p_load(c):
                chunks = []
                for k in range(n_chunks):
                    sk = src_pool.tile([P, cn, D], f32r, tag=f"s{k}")
                    nc.sync.dma_start(
                        out=sk,
                        in_=src_t[c, :, k * cn : (k + 1) * cn, :].bitcast(f32r),
                    )
                    chunks.append(sk)
                st[c] = {"chunks": chunks}

            def s_mm(c, n):
                return st[c]["chunks"][n // cn][:, n % cn, :]

            def s_of(c, n):
                return s_mm(c, n).bitcast(f32)

            def p_sq(c):
                # squares n = 0..7 on ACT; n = 8 rides on DVE (p_sqd) to
                # balance the engines (ACT accum reads cost ~280ns each).
                ssq = small_pool.tile([P, N], f32, tag="ssq")
                sq = scr_pool.tile([P, D], f32, tag="sq")
                for n in range(N - 1):
                    nc.scalar.activation(
                        out=sq,
                        in_=s_of(c, n),
                        func=Act.Square,
                        accum_out=ssq[:, n : n + 1],
                    )
                st[c]["ssq"] = ssq

            def p_sqd(c):
                n = N - 1
                nc.vector.scalar_tensor_tensor(
                    out=st[c]["prod"],
                    in0=s_of(c, n),
                    scalar=0.0,
                    in1=s_of(c, n),
                    op0=Alu.bypass,
                    op1=Alu.mult,
                    accum_out=st[c]["ssq"][:, n : n + 1],
                )

            def p_ln(c):
                # rsq = (ssq/D + eps)^(-1/2) via Exp(-0.5 * Ln(x))
                rsq = small_pool.tile([P, N], f32, tag="rsq")
                nc.scalar.activation(
                    out=rsq, in_=st[c]["ssq"], func=Act.Ln, scale=1.0 / D, bias=eps_sb
                )
                nc.scalar.activation(out=rsq, in_=rsq, func=Act.Exp, scale=-0.5)
                st[c]["rsq"] = rsq

            def p_dot(c, lo, hi):
                if lo == 0:
                    st[c]["dot"] = small_pool.tile([P, N], f32, tag="dot", name="dot")
                    st[c]["prod"] = scr_pool.tile([P, D], f32, tag="prod", name="prod")
                dot, prod = st[c]["dot"], st[c]["prod"]
                for n in range(lo, hi):
                    nc.vector.scalar_tensor_tensor(
                        out=prod,
                        in0=s_of(c, n),
                        scalar=0.0,
                        in1=w_sb,
                        op0=Alu.bypass,
                        op1=Alu.mult,
                        accum_out=dot[:, n : n + 1],
                    )

            def p_score(c):
                # negscore = -(dot * rsq); negmax = -max(score)
                negscore = small_pool.tile([P, N], f32, tag="score")
                nc.vector.scalar_tensor_tensor(
                    out=negscore,
                    in0=st[c]["dot"],
                    scalar=-1.0,
                    in1=st[c]["rsq"],
                    op0=Alu.mult,
                    op1=Alu.mult,
                )
                nmx = small_pool.tile([P, 1], f32, tag="nmx")
                nc.vector.tensor_reduce(
                    out=nmx, in_=negscore, axis=Ax.X, op=Alu.min
                )
                st[c]["score"], st[c]["nmx"] = negscore, nmx

            def p_e(c):
                # e = Exp(score - max) = Exp(-negscore + negmax); unnormalized
                # — the 1/sum_e normalization rides on the drain's scale.
                # accum_out gives sum_e for free.
                e = small_pool.tile([P, N], f32, tag="e")
                sume = small_pool.tile([P, 1], f32, tag="sume")
                nc.scalar.activation(
                    out=e,
                    in_=st[c]["score"],
                    func=Act.Exp,
                    scale=-1.0,
                    bias=st[c]["nmx"],
                    accum_out=sume,
                )
                st[c]["e"], st[c]["sume"] = e, sume

            def p_sums(c):
                rs = small_pool.tile([P, 1], f32, tag="rs")
                nc.vector.reciprocal(out=rs, in_=st[c]["sume"])
                st[c]["rs"] = rs

            def p_dg(c):
                # dg[:, n, :] = I * e[:, n] = diag(e_n) — ONE broadcast
                # tensor_tensor: in0 = I broadcast over n (stride-0), in1 =
                # e broadcast over the 128 columns (stride-0 innermost).
                # f32r so the PE consumes it directly.  (gpsimd was tried
                # here and steals DVE SBUF-port bandwidth ~1:1 — keep on DVE.)
                dg = diag_pool.tile([P, N, P], f32r, tag="dg")
                i_sb_, e_ = i_sb, st[c]["e"]
                i_b = bass.AP(
                    tensor=i_sb_.tensor,
                    offset=i_sb_.offset,
                    ap=[i_sb_.ap[0], [0, N], i_sb_.ap[1]],
                )
                e_b = bass.AP(
                    tensor=e_.tensor,
                    offset=e_.offset,
                    ap=[e_.ap[0], [e_.ap[1][0], N], [0, P]],
                )
                nc.vector.tensor_mul(dg, i_b, e_b)
                st[c]["dg"] = dg

            def p_mm(c):
                hp = psum_pool.tile([P, D], f32, tag="hp")
                dg = st[c]["dg"]
                for n in range(N):
                    nc.tensor.matmul(
                        hp,
                        dg[:, n, :],
                        s_mm(c, n),
                        start=(n == 0),
                        stop=(n == N - 1),
                    )
                st[c]["hp"] = hp

            def p_drain(c):
                # h = hp / sum_e: the PSUM->SBUF move doubles as the softmax
                # normalization (per-partition scale).
                hs = out_pool.tile([P, D], f32, tag="hs")
                nc.scalar.activation(
                    out=hs, in_=st[c]["hp"], func=Act.Copy, scale=st[c]["rs"]
                )
                st[c]["hs"] = hs

            def p_out(c):
                nc.sync.dma_start(out=out_t[c], in_=st[c]["hs"])
                del st[c]  # release python refs; pool rotation handles bufs

            # Software-pipelined emission (see module docstring).  Loads run
            # 2 tiles ahead of their consumers; the out-DMA trails its drain
            # by a full tile so the SP queue never parks on a compute sem.
            for c in range(ntiles + 6):
                if c - 6 >= 0 and c - 6 < ntiles:
                    p_out(c - 6)
                if c < ntiles:
                    p_load(c)
                x = c - 3
                if 0 <= x < ntiles:
                    p_ln(x)
                x = c - 4
                if 0 <= x < ntiles:
                    p_e(x)
                x = c - 2
                if 0 <= x < ntiles:
                    p_dot(x, 0, 1)
                x = c - 3
                if 0 <= x < ntiles:
                    p_score(x)
                x = c - 2
                if 0 <= x < ntiles:
                    p_dot(x, 1, N)
                    p_sq(x)
                    p_sqd(x)
                x = c - 4
                if 0 <= x < ntiles:
                    p_sums(x)
                    p_dg(x)
                    p_mm(x)
                x = c - 5
                if 0 <= x < ntiles:
                    p_drain(x)

    nc.compile()
    return nc


def _get_nc(t_len=T):
    key = (t_len,)
    if key not in _CACHE:
        _CACHE[key] = _build_bass(t_len)
    return _CACHE[key]


def _make_in_maps(sources, queries, layer_idx):
    sources = np.ascontiguousarray(np.asarray(sources, dtype=np.float32))
    queries = np.asarray(queries, dtype=np.float32)
    w = queries[int(layer_idx)]
    w_rep = np.ascontiguousarray(np.broadcast_to(w[None, :], (P, D)).astype(np.float32))
    idn = np.eye(P, dtype=np.float32)
    return [
        {"src": np.ascontiguousarray(sources[b]), "wq": w_rep, "idn": idn}
        for b in range(sources.shape[0])
    ]


def kernel(sources, queries, layer_idx):
    from concourse.bass_utils import run_bass_kernel_spmd

    nc = _get_nc()
    in_maps = _make_in_maps(sources, queries, layer_idx)
    res = run_bass_kernel_spmd(nc, in_maps, core_ids=list(range(NCORES)))
    return np.stack([res.results[b]["out"] for b in range(NCORES)], axis=0)
